# revision 14
# baseline (speedup 1.0000x reference)
"""Trainium2 Bass kernel for nn_DifferentiableConstructor (gnn_message_passing).

Strategy (8 cores, SPMD — identical NEFF, per-core input data):
  * Host: embedding gather + positional encoding -> x0T [D,S] (replicated).
  * Device (every core): 2-layer transformer backbone in transposed layout
    [D partitions, S free]; node head; then its 64 source-row slice of the
    (full SxS grid) edge computation:
      - edge scorer:  hs = gelu(C + rowvec_s)  with C = W1b^T xT + w1c x (-t/S)
        shared across rows, rowvec per-partition bias; logits via PE with hs
        as the stationary operand -> t-partitioned logit columns.
      - gumbel-sigmoid gates computed in the same t-partitioned layout
        (u pre-laid-out by host).
      - gated edge features: te-branch in doubled [2x64f, S] layout, bias add,
        PE transpose, one broadcast multiply by the gate columns, DMA out.
  * Host: concatenate 8 slices, drop the diagonal, emit static edge_index.
"""
import numpy as np

S = 512
V = 32000
D = 128
H = 4
DH = 32
L = 2
DN = 64
DE = 64
DF = 4 * D
E = S * S - S
NC = 8
R = S // NC          # 64 source rows per core
NQ = 4               # quarters per core
RQ = R // NQ         # 16 rows per quarter

_SRC = np.repeat(np.arange(S), S)
_TGT = np.tile(np.arange(S), S)
_KEEP = _SRC != _TGT
_EDGE_INDEX = np.stack([_SRC[_KEEP], _TGT[_KEEP]]).astype(np.int32)

_BUILT = {}


def _pos_encoding():
    pos = np.arange(S, dtype=np.float32)[:, None]
    div = np.exp(np.arange(0, D, 2, dtype=np.float32) * (-np.log(10000.0) / D))
    pe = np.zeros((S, D), dtype=np.float32)
    pe[:, 0::2] = np.sin(pos * div)
    pe[:, 1::2] = np.cos(pos * div)
    return pe


def build_module(gelu_mode="hw"):
    """Build the Bass module (same program for all 8 cores).

    gelu_mode="sim" replaces Gelu with Identity (CoreSim does not implement
    Gelu); the numpy model in sim tests does the same.
    """
    import concourse.bass as bass
    import concourse.tile as tile
    from concourse import bacc, mybir

    f32 = mybir.dt.float32
    fr = mybir.dt.float32r
    AF = mybir.ActivationFunctionType
    OP = mybir.AluOpType
    GELU = AF.Identity if gelu_mode == "sim" else AF.Gelu

    nc = bacc.Bacc("TRN2", target_bir_lowering=False, debug=False)

    def din(name, shape, dt=f32):
        return nc.dram_tensor(name, shape, dt, kind="ExternalInput")

    x0T = din("x0T", [D, S], fr)
    Wq, Wk, Wv, bq, bk, Wo, bo = [], [], [], [], [], [], []
    g1, c1, Wf1, bf1c, Wf2, bf2, g2, c2 = [], [], [], [], [], [], [], []
    for l in range(L):
        Wq.append(din(f"Wq{l}", [D, D], fr))
        Wk.append(din(f"Wk{l}", [D, D], fr))
        Wv.append(din(f"Wv{l}", [D, D], fr))
        bq.append(din(f"bq{l}", [D, 1]))
        bk.append(din(f"bk{l}", [D, 1]))
        Wo.append(din(f"Wo{l}", [DH, H, D], fr))
        bo.append(din(f"bo{l}", [D, 1]))
        g1.append(din(f"g1{l}", [D, 1]))
        c1.append(din(f"c1{l}", [D, 1]))
        Wf1.append(din(f"Wf1{l}", [D, DF], fr))
        bf1c.append(din(f"bf1c{l}", [D, 4]))
        Wf2.append(din(f"Wf2{l}", [DF, D], fr))
        bf2.append(din(f"bf2{l}", [D, 1]))
        g2.append(din(f"g2{l}", [D, 1]))
        c2.append(din(f"c2{l}", [D, 1]))
    nodeW = din("nodeW", [D, DN], fr)
    nodebB = din("nodebB", [D, DN])
    W1a = din("W1a", [D, D], fr)
    W1b = din("W1b", [D, D], fr)
    w1c = din("w1c", [1, D], fr)
    b1 = din("b1", [D, 1])
    w2 = din("w2", [D, 1])
    teWa = din("teWa", [D, DE], fr)
    teWb2 = din("teWb2", [D, 2 * DE], fr)
    tewc = din("tewc", [1, DE], fr)
    tewc2 = din("tewc2", [1, 2 * DE], fr)
    teb2 = din("teb2", [2 * DE, 1])
    b2col = din("b2col", [D, 1])
    negtvec = din("negtvec", [1, S], fr)
    identD = din("identD", [D, D], fr)
    ones_col = din("ones_col", [D, 1], fr)
    ones_row = din("ones_row", [1, D], fr)
    invD = din("invD", [D, 1], fr)
    # per-core data
    EselT = din("EselT", [S, R], fr)
    EselTe = din("EselTe", [S, R // 2], fr)
    EselTo = din("EselTo", [S, R // 2], fr)
    svec = din("svec", [1, R], fr)
    svec_e = din("svec_e", [1, R // 2], fr)
    svec_o = din("svec_o", [1, R // 2], fr)
    u_lay = din("u_lay", [NQ, D, R])

    node_out = nc.dram_tensor("node_out", [S, DN], f32, kind="ExternalOutput")
    feats_out = nc.dram_tensor("feats_out", [R, S, DE], f32, kind="ExternalOutput")
    gatesT_out = nc.dram_tensor("gatesT_out", [NQ, D, R], f32, kind="ExternalOutput")

    SCALE = float(1.0 / np.sqrt(DH))

    def r(ap):
        return ap

    with tile.TileContext(nc) as tc:
        with (
            nc.allow_low_precision(reason="fp32r operand chain for PE fast fp32"),
            tc.tile_pool(name="singles", bufs=1) as sg,
            tc.tile_pool(name="work", bufs=3) as wk,
            tc.tile_pool(name="hsp", bufs=3) as hsp,
            tc.tile_pool(name="gts", bufs=2) as gtsp,
            tc.tile_pool(name="fez", bufs=3) as fez,
            tc.tile_pool(name="pmm", bufs=2, space="PSUM") as pmm,
            tc.tile_pool(name="pacc", bufs=2, space="PSUM") as pacc,
            tc.tile_pool(name="psml", bufs=3, space="PSUM") as psml,
        ):
            # ---------- load constants/weights ----------
            def load(dram, shape, rearr=None, **axes):
                t = sg.tile(shape, dram.dtype, tag=dram.name)
                src = dram[:]
                if rearr is not None:
                    src = src.rearrange(rearr, **axes)
                nc.sync.dma_start(t[:], src)
                return t

            zcol = sg.tile([D, 1], f32, tag="zcol")
            nc.vector.memset(zcol[:], 0.0)
            ecol = sg.tile([1, 1], f32, tag="ecol")
            nc.vector.memset(ecol[:], 1e-5)
            xT = sg.tile([D, S], fr, tag="xT0")
            nc.sync.dma_start(xT[:], x0T[:])
            sWq = [load(Wq[l], [D, D]) for l in range(L)]
            sWk = [load(Wk[l], [D, D]) for l in range(L)]
            sWv = [load(Wv[l], [D, D]) for l in range(L)]
            sbq = [load(bq[l], [D, 1]) for l in range(L)]
            sbk = [load(bk[l], [D, 1]) for l in range(L)]
            sWo = [load(Wo[l], [DH, H, D]) for l in range(L)]
            sbo = [load(bo[l], [D, 1]) for l in range(L)]
            sg1 = [load(g1[l], [D, 1]) for l in range(L)]
            sc1 = [load(c1[l], [D, 1]) for l in range(L)]
            sWf1 = [load(Wf1[l], [D, DF]) for l in range(L)]
            sbf1 = [load(bf1c[l], [D, 4]) for l in range(L)]
            sWf2 = [load(Wf2[l], [D, 4, D], "(c p) n -> p c n", p=D) for l in range(L)]
            sbf2 = [load(bf2[l], [D, 1]) for l in range(L)]
            sg2 = [load(g2[l], [D, 1]) for l in range(L)]
            sc2 = [load(c2[l], [D, 1]) for l in range(L)]
            snodeW = load(nodeW, [D, DN])
            snodeb = load(nodebB, [D, DN])
            sW1a = load(W1a, [D, D])
            sW1b = load(W1b, [D, D])
            sw1c = load(w1c, [1, D])
            sb1 = load(b1, [D, 1])
            sw2 = load(w2, [D, 1])
            steWa = load(teWa, [D, DE])
            steWb2 = load(teWb2, [D, 2 * DE])
            stewc = load(tewc, [1, DE])
            stewc2 = load(tewc2, [1, 2 * DE])
            steb2 = load(teb2, [2 * DE, 1])
            sb2c = load(b2col, [D, 1])
            sneg = load(negtvec, [1, S])
            sid = load(identD, [D, D])
            sones = load(ones_col, [D, 1])
            sonesr = load(ones_row, [1, D])
            sinvD = load(invD, [D, 1])
            sEselT = load(EselT, [D, 4, R], "(c p) j -> p c j", p=D)
            sEselTe = load(EselTe, [D, 4, R // 2], "(c p) j -> p c j", p=D)
            sEselTo = load(EselTo, [D, 4, R // 2], "(c p) j -> p c j", p=D)
            ssvec = load(svec, [1, R])
            ssvec_e = load(svec_e, [1, R // 2])
            ssvec_o = load(svec_o, [1, R // 2])
            su = load(u_lay, [D, NQ, R], "q p j -> p q j")

            # ---------- backbone ----------
            for l in range(L):
                q_ps = pmm.tile([D, S], f32, tag="mm")
                nc.tensor.matmul(q_ps[:], r(sWq[l][:]), r(xT[:]), start=True, stop=True)
                qT = wk.tile([D, S], fr, tag="qT")
                nc.vector.tensor_scalar_add(qT[:], q_ps[:], sbq[l][:])
                k_ps = pmm.tile([D, S], f32, tag="mm")
                nc.tensor.matmul(k_ps[:], r(sWk[l][:]), r(xT[:]), start=True, stop=True)
                kT = wk.tile([D, S], fr, tag="kT")
                nc.vector.tensor_scalar_add(kT[:], k_ps[:], sbk[l][:])
                v_sb = wk.tile([D, 4, D], fr, tag="v")
                for j in range(4):
                    v_ps = psml.tile([D, D], f32, tag="sm")
                    nc.tensor.matmul(v_ps[:], xT[:, j * D:(j + 1) * D],
                                     sWv[l][:], start=True, stop=True)
                    nc.vector.tensor_copy(v_sb[:, j, :], v_ps[:])

                pj_ps = pacc.tile([D, S], f32, tag="acc")
                for h in range(H):
                    den_ps = psml.tile([1, S], f32, tag="sm")
                    oTh_ps = psml.tile([DH, S], f32, tag="sm")
                    for kc in range(4):
                        scT = pmm.tile([D, S], f32, tag="mm")
                        nc.tensor.matmul(
                            scT[:],
                            r(kT[32 * h:32 * h + 32, kc * D:(kc + 1) * D]),
                            r(qT[32 * h:32 * h + 32, :]),
                            start=True, stop=True, tile_position=(32 * h, 0))
                        P = hsp.tile([D, S], fr, tag="P")
                        nc.scalar.activation(P[:], scT[:], AF.Exp, bias=zcol[:], scale=SCALE)
                        nc.tensor.matmul(oTh_ps[:],
                                         r(v_sb[:, kc, 32 * h:32 * h + 32]),
                                         r(P[:]), start=(kc == 0), stop=(kc == 3))
                        nc.tensor.matmul(den_ps[:], r(sones[:]), r(P[:]),
                                         start=(kc == 0), stop=(kc == 3))
                    rd = wk.tile([1, S], fr, tag="rd")
                    nc.vector.reciprocal(rd[:], den_ps[:])
                    RDh_ps = psml.tile([DH, S], f32, tag="sm")
                    nc.tensor.matmul(RDh_ps[:], sonesr[0:1, 0:DH], rd[:],
                                     start=True, stop=True)
                    oTh_sb = wk.tile([DH, S], fr, tag="oTh_sb")
                    nc.vector.tensor_copy(oTh_sb[:], oTh_ps[:])
                    oTn = wk.tile([DH, S], fr, tag="oTn")
                    nc.vector.tensor_mul(oTn[:], oTh_sb[:], RDh_ps[:])
                    nc.tensor.matmul(pj_ps[:], sWo[l][:, h, :], oTn[:],
                                     start=(h == 0), stop=(h == 3))
                xa = wk.tile([D, S], fr, tag="xa")
                nc.vector.tensor_scalar_add(xa[:], pj_ps[:], sbo[l][:])
                nc.vector.tensor_add(xa[:], xa[:], xT[:])

                def layernorm(xin, gcol, ccol, outtag):
                    sq = wk.tile([D, S], fr, tag="sq")
                    nc.scalar.activation(sq[:], xin[:], AF.Square, bias=zcol[:])
                    mu_ps = psml.tile([1, S], f32, tag="sm")
                    nc.tensor.matmul(mu_ps[:], r(sinvD[:]), r(xin[:]), start=True, stop=True)
                    m2_ps = psml.tile([1, S], f32, tag="sm")
                    nc.tensor.matmul(m2_ps[:], r(sinvD[:]), r(sq[:]), start=True, stop=True)
                    mu = wk.tile([1, S], fr, tag="mu_sb")
                    nc.vector.tensor_copy(mu[:], mu_ps[:])
                    var = wk.tile([1, S], fr, tag="var")
                    nc.vector.tensor_mul(var[:], mu[:], mu[:])
                    nc.vector.tensor_sub(var[:], m2_ps[:], var[:])
                    sd = wk.tile([1, S], fr, tag="sd")
                    nc.scalar.activation(sd[:], var[:], AF.Sqrt, bias=ecol[:])
                    rstd = wk.tile([1, S], fr, tag="rstd")
                    nc.vector.reciprocal(rstd[:], sd[:])
                    MB = pmm.tile([D, S], f32, tag="mm")
                    nc.tensor.matmul(MB[:], r(sonesr[:]), r(mu[:]), start=True, stop=True)
                    RB = pmm.tile([D, S], f32, tag="mm")
                    nc.tensor.matmul(RB[:], r(sonesr[:]), r(rstd[:]), start=True, stop=True)
                    xout = wk.tile([D, S], fr, tag=outtag)
                    nc.vector.tensor_sub(xout[:], xin[:], MB[:])
                    nc.vector.tensor_mul(xout[:], xout[:], RB[:])
                    nc.vector.tensor_scalar(xout[:], xout[:], gcol[:], ccol[:],
                                            OP.mult, OP.add)
                    return xout

                xm = layernorm(xa, sg1[l], sc1[l], "xm")

                f2_ps = pacc.tile([D, S], f32, tag="acc")
                for j in range(4):
                    h_ps = pmm.tile([D, S], f32, tag="mm")
                    nc.tensor.matmul(h_ps[:], r(sWf1[l][:, j * D:(j + 1) * D]),
                                     r(xm[:]), start=True, stop=True)
                    hT = hsp.tile([D, S], fr, tag="hT")
                    nc.scalar.activation(hT[:], h_ps[:], GELU, bias=sbf1[l][:, j:j + 1])
                    nc.tensor.matmul(f2_ps[:], r(sWf2[l][:, j, :]), r(hT[:]),
                                     start=(j == 0), stop=(j == 3))
                xb = wk.tile([D, S], fr, tag="xb")
                nc.vector.tensor_scalar_add(xb[:], f2_ps[:], sbf2[l][:])
                nc.vector.tensor_add(xb[:], xb[:], xm[:])
                xT = layernorm(xb, sg2[l], sc2[l], "xT_out")

            # ---------- node head ----------
            for j in range(4):
                n_ps = psml.tile([D, DN], f32, tag="sm")
                nc.tensor.matmul(n_ps[:], xT[:, j * D:(j + 1) * D], snodeW[:],
                                 start=True, stop=True)
                n_sb = wk.tile([D, DN], f32, tag="n_sb")
                nc.vector.tensor_add(n_sb[:], n_ps[:], snodeb[:])
                nc.sync.dma_start(node_out[j * D:(j + 1) * D, :], n_sb[:])

            # ---------- edge setup ----------
            C_ps = pmm.tile([D, S], f32, tag="mm")
            nc.tensor.matmul(C_ps[:], r(sW1b[:]), r(xT[:]), start=True, stop=False)
            nc.tensor.matmul(C_ps[:], r(sw1c[:]), r(sneg[:]), start=False, stop=True)
            C_sb = sg.tile([D, S], f32, tag="C_sb")
            nc.scalar.copy(C_sb[:], C_ps[:])

            CT2_ps = pmm.tile([D, S], f32, tag="mm")
            nc.tensor.matmul(CT2_ps[:], r(steWb2[:]), r(xT[:]), start=True, stop=False)
            nc.tensor.matmul(CT2_ps[:], r(stewc2[:]), r(sneg[:]), start=False, stop=True)
            CT2_sb = sg.tile([D, S], f32, tag="CT2_sb")
            nc.scalar.copy(CT2_sb[:], CT2_ps[:])

            AWsc = sg.tile([D, 4, D], fr, tag="AWsc")
            ATe = sg.tile([D, 4, DE], fr, tag="ATe")
            for tcn in range(4):
                aw_ps = psml.tile([D, D], f32, tag="sm")
                nc.tensor.matmul(aw_ps[:], xT[:, tcn * D:(tcn + 1) * D], sW1a[:],
                                 start=True, stop=True)
                nc.vector.tensor_copy(AWsc[:, tcn, :], aw_ps[:])
                at_ps = psml.tile([D, DE], f32, tag="sm")
                nc.tensor.matmul(at_ps[:], xT[:, tcn * D:(tcn + 1) * D], steWa[:],
                                 start=True, stop=True)
                nc.vector.tensor_copy(ATe[:, tcn, :], at_ps[:])

            rvT_ps = psml.tile([R, D], f32, tag="sm")
            for tcn in range(4):
                nc.tensor.matmul(rvT_ps[:], sEselT[:, tcn, :], AWsc[:, tcn, :],
                                 start=(tcn == 0), stop=False)
            nc.tensor.matmul(rvT_ps[:], ssvec[:], sw1c[:], start=False, stop=True)
            rvT_sb = wk.tile([R, D], fr, tag="rvT_sb")
            nc.vector.tensor_copy(rvT_sb[:], rvT_ps[:])
            rvsc_ps = psml.tile([D, R], fr, tag="sm")
            nc.tensor.matmul(rvsc_ps[:], rvT_sb[:], sid[0:R, 0:R],
                             is_transpose=True, start=True, stop=True)
            RVsc = sg.tile([D, R], f32, tag="RVsc")
            nc.vector.tensor_scalar_add(RVsc[:], rvsc_ps[:], sb1[:])

            rvte2_raw = sg.tile([2 * DE, R // 2], f32, tag="rvte2_raw")
            for half, (esel, sv) in enumerate(
                    [(sEselTe, ssvec_e), (sEselTo, ssvec_o)]):
                rvh_ps = psml.tile([R // 2, DE], f32, tag="sm")
                for tcn in range(4):
                    nc.tensor.matmul(rvh_ps[:], esel[:, tcn, :], ATe[:, tcn, :],
                                     start=(tcn == 0), stop=False)
                nc.tensor.matmul(rvh_ps[:], sv[:], stewc[:], start=False, stop=True)
                rvh_sb = wk.tile([R // 2, DE], fr, tag="rvh_sb")
                nc.vector.tensor_copy(rvh_sb[:], rvh_ps[:])
                rvt_ps = psml.tile([DE, R // 2], fr, tag="sm")
                nc.tensor.matmul(rvt_ps[:], rvh_sb[:], sid[0:R // 2, 0:R // 2],
                                 is_transpose=True, start=True, stop=True)
                rvt_sb = wk.tile([DE, R // 2], f32, tag="rvt_sb")
                nc.vector.tensor_copy(rvt_sb[:], rvt_ps[:])
                nc.sync.dma_start(rvte2_raw[half * DE:(half + 1) * DE, :], rvt_sb[:])
            RVte2 = sg.tile([2 * DE, R // 2], f32, tag="RVte2")
            nc.vector.tensor_scalar_add(RVte2[:], rvte2_raw[:], steb2[:])

            # ---------- per-quarter: logits -> gates -> gated features ----------
            import concourse.bass as bassmod
            for q in range(NQ):
                LGT_full = pacc.tile([D, S], f32, tag="acc", name="LGT")
                LGT = LGT_full[:, 0:R]
                for rr in range(RQ):
                    sl = q * RQ + rr
                    hs = hsp.tile([D, S], f32, tag="hs")
                    nc.scalar.activation(hs[:], C_sb[:], GELU,
                                         bias=RVsc[:, sl:sl + 1])
                    for tcn in range(4):
                        nc.tensor.matmul(LGT[:, rr * 4 + tcn:rr * 4 + tcn + 1],
                                         hs[:, tcn * D:(tcn + 1) * D], sw2[:],
                                         start=True, stop=True)
                # gates for this quarter, in [t-partition, (row,chunk)] layout
                gt = gtsp.tile([D, R], f32, tag="gt_tmp")
                nc.vector.tensor_scalar(gt[:], su[:, q, :], 1e-8, 1.0 - 1e-8,
                                        OP.max, OP.min)
                nc.scalar.activation(gt[:], gt[:], AF.Ln, bias=zcol[:])
                nc.scalar.activation(gt[:], gt[:], AF.Ln, bias=zcol[:], scale=-1.0)
                nc.vector.tensor_scalar(gt[:], gt[:], -1.0, sb2c[:],
                                        OP.mult, OP.add)
                nc.vector.tensor_add(gt[:], gt[:], LGT[:])
                GTS = gtsp.tile([D, R], f32, tag="GTS")
                nc.scalar.activation(GTS[:], gt[:], AF.Sigmoid, bias=zcol[:])
                nc.sync.dma_start(gatesT_out[q, :, :], GTS[:])

                for pp in range(RQ // 2):
                    p_glob = q * (RQ // 2) + pp
                    re = 2 * pp          # even row index within quarter
                    t1 = fez.tile([2 * DE, S], fr, tag="t1")
                    nc.vector.tensor_scalar_add(t1[:], CT2_sb[:],
                                                RVte2[:, p_glob:p_glob + 1])
                    tr_ps = psml.tile([D, 4, D], fr, tag="sm")
                    for tcn in range(4):
                        nc.tensor.matmul(tr_ps[:, tcn, :],
                                         t1[:, tcn * D:(tcn + 1) * D], sid[:],
                                         is_transpose=True, start=True, stop=True)
                    feats = fez.tile([D, 2, 4, DE], f32, tag="feats")
                    gts_ap = GTS[:]
                    bcast = bassmod.AP(
                        tensor=gts_ap.tensor,
                        offset=gts_ap.offset + re * 4,
                        ap=[list(gts_ap.ap[0]), [4, 2], [1, 4], [0, DE]],
                    )
                    nc.vector.tensor_tensor(
                        feats[:],
                        tr_ps[:].rearrange("p c (h f) -> p h c f", h=2),
                        bcast, OP.mult)
                    dst = feats_out[2 * p_glob:2 * p_glob + 2, :, :].rearrange(
                        "h (c p) f -> p h c f", p=D)
                    nc.sync.dma_start(dst, feats[:])

    nc.compile()
    return nc


def _host_prep(token_ids, u, emb, Wqkv, bqkv, Wo, bo, ln1_g, ln1_b, Wff1, bff1,
               Wff2, bff2, ln2_g, ln2_b, node_W, node_b, sc_W1, sc_b1, sc_W2,
               sc_b2, te_W, te_b):
    f = np.float32
    a = lambda x: np.ascontiguousarray(x, dtype=f)
    x0 = emb[np.asarray(token_ids).astype(np.int64)].astype(f) + _pos_encoding()
    shared = {"x0T": a(x0.T)}
    for l in range(L):
        shared[f"Wq{l}"] = a(Wqkv[l][:, 0:D])
        shared[f"Wk{l}"] = a(Wqkv[l][:, D:2 * D])
        shared[f"Wv{l}"] = a(Wqkv[l][:, 2 * D:3 * D])
        shared[f"bq{l}"] = a(bqkv[l][0:D].reshape(D, 1))
        shared[f"bk{l}"] = a(bqkv[l][D:2 * D].reshape(D, 1))
        shared[f"Wo{l}"] = a(Wo[l].reshape(H, DH, D).transpose(1, 0, 2))
        bv = bqkv[l][2 * D:3 * D]
        shared[f"bo{l}"] = a((bo[l] + bv @ Wo[l]).reshape(D, 1))
        shared[f"g1{l}"] = a(ln1_g[l].reshape(D, 1))
        shared[f"c1{l}"] = a(ln1_b[l].reshape(D, 1))
        shared[f"Wf1{l}"] = a(Wff1[l])
        shared[f"bf1c{l}"] = a(bff1[l].reshape(4, D).T)
        shared[f"Wf2{l}"] = a(Wff2[l])
        shared[f"bf2{l}"] = a(bff2[l].reshape(D, 1))
        shared[f"g2{l}"] = a(ln2_g[l].reshape(D, 1))
        shared[f"c2{l}"] = a(ln2_b[l].reshape(D, 1))
    shared["nodeW"] = a(node_W)
    shared["nodebB"] = a(np.broadcast_to(node_b, (D, DN)))
    shared["W1a"] = a(sc_W1[0:D])
    shared["W1b"] = a(sc_W1[D:2 * D])
    shared["w1c"] = a(sc_W1[2 * D:2 * D + 1])
    shared["b1"] = a(sc_b1.reshape(D, 1))
    shared["w2"] = a(sc_W2)
    shared["teWa"] = a(te_W[0:D])
    teWb = te_W[D:2 * D]
    shared["teWb2"] = a(np.concatenate([teWb, teWb], axis=1))
    shared["tewc"] = a(te_W[2 * D:2 * D + 1])
    shared["tewc2"] = a(np.concatenate([te_W[2 * D:2 * D + 1]] * 2, axis=1))
    shared["teb2"] = a(np.concatenate([te_b, te_b]).reshape(2 * DE, 1))
    shared["b2col"] = a(np.full((D, 1), np.asarray(sc_b2).reshape(-1)[0]))
    shared["negtvec"] = a((-np.arange(S, dtype=f) / S).reshape(1, S))
    shared["identD"] = a(np.eye(D))
    shared["ones_col"] = a(np.ones((D, 1)))
    shared["ones_row"] = a(np.ones((1, D)))
    shared["invD"] = a(np.full((D, 1), 1.0 / D))

    u_grid = np.full((S, S), 0.5, dtype=f)
    u_grid[_SRC[_KEEP], _TGT[_KEEP]] = np.asarray(u, dtype=f)

    per_core = []
    for c in range(NC):
        s0 = c * R
        esel = np.zeros((S, R), dtype=f)
        esel[s0 + np.arange(R), np.arange(R)] = 1.0
        ug = u_grid[s0:s0 + R].reshape(NQ, RQ, 4, D)
        pc = {
            "EselT": esel,
            "EselTe": a(esel[:, 0::2]),
            "EselTo": a(esel[:, 1::2]),
            "svec": a(((s0 + np.arange(R)) / S).reshape(1, R)),
            "svec_e": a(((s0 + np.arange(0, R, 2)) / S).reshape(1, R // 2)),
            "svec_o": a(((s0 + np.arange(1, R, 2)) / S).reshape(1, R // 2)),
            "u_lay": a(ug.transpose(0, 3, 1, 2).reshape(NQ, D, R)),
        }
        per_core.append(pc)
    return shared, per_core


def _assemble(results):
    node_features = np.asarray(results[0]["node_out"], dtype=np.float32)
    feats_grid = np.concatenate(
        [np.asarray(r["feats_out"]) for r in results], axis=0)
    edge_feats = np.ascontiguousarray(
        feats_grid.reshape(S * S, DE)[_KEEP]).astype(np.float32)
    gates_rows = []
    for rres in results:
        gt = np.asarray(rres["gatesT_out"])  # [NQ, D, R]
        gates_rows.append(
            gt.reshape(NQ, D, RQ, 4).transpose(0, 2, 3, 1).reshape(R, S))
    gates_grid = np.concatenate(gates_rows, axis=0)
    edge_gates = np.ascontiguousarray(
        gates_grid.reshape(S * S)[_KEEP]).astype(np.float32)
    return node_features, edge_feats, _EDGE_INDEX.copy(), edge_gates


def kernel(**inputs):
    from concourse.bass_utils import run_bass_kernel_spmd

    if "hw" not in _BUILT:
        _BUILT["hw"] = build_module("hw")
    nc = _BUILT["hw"]
    shared, per_core = _host_prep(**inputs)
    in_maps = [{**shared, **pc} for pc in per_core]
    res = run_bass_kernel_spmd(nc, in_maps, core_ids=list(range(NC)))
    return _assemble(res.results)


# revision 16
# speedup vs baseline: 1.1772x; 1.1772x over previous
"""Trainium2 Bass kernel for nn_DifferentiableConstructor (gnn_message_passing).

Strategy (8 cores, SPMD — identical NEFF, per-core input data):
  * Host: embedding gather + positional encoding -> x0T [D,S] (replicated).
  * Device (every core): 2-layer transformer backbone in transposed layout
    [D partitions, S free]; node head; then its 64 source-row slice of the
    (full SxS grid) edge computation:
      - edge scorer:  hs = gelu(C + rowvec_s)  with C = W1b^T xT + w1c x (-t/S)
        shared across rows, rowvec per-partition bias; logits via PE with hs
        as the stationary operand -> t-partitioned logit columns.
      - gumbel-sigmoid gates computed in the same t-partitioned layout
        (u pre-laid-out by host).
      - gated edge features: te-branch in doubled [2x64f, S] layout, bias add,
        PE transpose, one broadcast multiply by the gate columns, DMA out.
  * Host: concatenate 8 slices, drop the diagonal, emit static edge_index.
"""
import numpy as np

S = 512
V = 32000
D = 128
H = 4
DH = 32
L = 2
DN = 64
DE = 64
DF = 4 * D
E = S * S - S
NC = 8
R = S // NC          # 64 source rows per core
NQ = 2               # row-batches per core
RQ = R // NQ         # 32 rows per batch

_SRC = np.repeat(np.arange(S), S)
_TGT = np.tile(np.arange(S), S)
_KEEP = _SRC != _TGT
_EDGE_INDEX = np.stack([_SRC[_KEEP], _TGT[_KEEP]]).astype(np.int32)

_BUILT = {}


def _pos_encoding():
    pos = np.arange(S, dtype=np.float32)[:, None]
    div = np.exp(np.arange(0, D, 2, dtype=np.float32) * (-np.log(10000.0) / D))
    pe = np.zeros((S, D), dtype=np.float32)
    pe[:, 0::2] = np.sin(pos * div)
    pe[:, 1::2] = np.cos(pos * div)
    return pe


def build_module(gelu_mode="hw"):
    """Build the Bass module (same program for all 8 cores).

    gelu_mode="sim" replaces Gelu with Identity (CoreSim does not implement
    Gelu); the numpy model in sim tests does the same.
    """
    import concourse.bass as bass
    import concourse.tile as tile
    from concourse import bacc, mybir

    f32 = mybir.dt.float32
    fr = mybir.dt.float32r
    f16 = mybir.dt.float16
    AF = mybir.ActivationFunctionType
    OP = mybir.AluOpType
    GELU = AF.Identity if gelu_mode == "sim" else AF.Gelu

    nc = bacc.Bacc("TRN2", target_bir_lowering=False, debug=False)

    def din(name, shape, dt=f32):
        return nc.dram_tensor(name, shape, dt, kind="ExternalInput")

    x0T = din("x0T", [D, S], fr)
    Wq, Wk, Wv, bq, bk, Wo, bo = [], [], [], [], [], [], []
    g1, c1, Wf1, bf1c, Wf2, bf2, g2, c2 = [], [], [], [], [], [], [], []
    for l in range(L):
        Wq.append(din(f"Wq{l}", [D, D], fr))
        Wk.append(din(f"Wk{l}", [D, D], fr))
        Wv.append(din(f"Wv{l}", [D, D], fr))
        bq.append(din(f"bq{l}", [D, 1]))
        bk.append(din(f"bk{l}", [D, 1]))
        Wo.append(din(f"Wo{l}", [DH, H, D], fr))
        bo.append(din(f"bo{l}", [D, 1]))
        g1.append(din(f"g1{l}", [D, 1]))
        c1.append(din(f"c1{l}", [D, 1]))
        Wf1.append(din(f"Wf1{l}", [D, DF], fr))
        bf1c.append(din(f"bf1c{l}", [D, 4]))
        Wf2.append(din(f"Wf2{l}", [DF, D], fr))
        bf2.append(din(f"bf2{l}", [D, 1]))
        g2.append(din(f"g2{l}", [D, 1]))
        c2.append(din(f"c2{l}", [D, 1]))
    nodeW = din("nodeW", [D, DN], fr)
    nodebB = din("nodebB", [D, DN])
    W1a = din("W1a", [D, D], fr)
    W1b = din("W1b", [D, D], fr)
    w1c = din("w1c", [1, D], fr)
    b1 = din("b1", [D, 1])
    w2 = din("w2", [D, 1], f16)
    identH = din("identH", [D, D], f16)
    teWa = din("teWa", [D, DE], fr)
    teWb2 = din("teWb2", [D, 2 * DE], fr)
    tewc = din("tewc", [1, DE], fr)
    tewc2 = din("tewc2", [1, 2 * DE], fr)
    teb2 = din("teb2", [2 * DE, 1])
    b2col = din("b2col", [D, 1])
    negtvec = din("negtvec", [1, S], fr)
    identD = din("identD", [D, D], fr)
    ones_col = din("ones_col", [D, 1], fr)
    ones_row = din("ones_row", [1, D], fr)
    invD = din("invD", [D, 1], fr)
    # per-core data
    EselT = din("EselT", [S, R], fr)
    EselTe = din("EselTe", [S, R // 2], fr)
    EselTo = din("EselTo", [S, R // 2], fr)
    svec = din("svec", [1, R], fr)
    svec_e = din("svec_e", [1, R // 2], fr)
    svec_o = din("svec_o", [1, R // 2], fr)
    u_lay = din("u_lay", [NQ, D, RQ * 4])

    node_out = nc.dram_tensor("node_out", [S, DN], f32, kind="ExternalOutput")
    feats_out = nc.dram_tensor("feats_out", [R, S, DE], f32, kind="ExternalOutput")
    gatesT_out = nc.dram_tensor("gatesT_out", [NQ, D, RQ * 4], f32, kind="ExternalOutput")

    SCALE = float(1.0 / np.sqrt(DH))

    def r(ap):
        return ap

    with tile.TileContext(nc) as tc:
        with (
            nc.allow_low_precision(reason="fp32r operand chain for PE fast fp32"),
            tc.tile_pool(name="singles", bufs=1) as sg,
            tc.tile_pool(name="work", bufs=3) as wk,
            tc.tile_pool(name="hsp", bufs=3) as hsp,
            tc.tile_pool(name="gts", bufs=2) as gtsp,
            tc.tile_pool(name="fez", bufs=3) as fez,
            tc.tile_pool(name="pmm", bufs=2, space="PSUM") as pmm,
            tc.tile_pool(name="pacc", bufs=2, space="PSUM") as pacc,
            tc.tile_pool(name="psml", bufs=3, space="PSUM") as psml,
        ):
            # ---------- load constants/weights ----------
            def load(dram, shape, rearr=None, **axes):
                t = sg.tile(shape, dram.dtype, tag=dram.name)
                src = dram[:]
                if rearr is not None:
                    src = src.rearrange(rearr, **axes)
                nc.sync.dma_start(t[:], src)
                return t

            zcol = sg.tile([D, 1], f32, tag="zcol")
            nc.vector.memset(zcol[:], 0.0)
            ecol = sg.tile([1, 1], f32, tag="ecol")
            nc.vector.memset(ecol[:], 1e-5)
            xT = sg.tile([D, S], fr, tag="xT0")
            nc.sync.dma_start(xT[:], x0T[:])
            sWq = [load(Wq[l], [D, D]) for l in range(L)]
            sWk = [load(Wk[l], [D, D]) for l in range(L)]
            sWv = [load(Wv[l], [D, D]) for l in range(L)]
            sbq = [load(bq[l], [D, 1]) for l in range(L)]
            sbk = [load(bk[l], [D, 1]) for l in range(L)]
            sWo = [load(Wo[l], [DH, H, D]) for l in range(L)]
            sbo = [load(bo[l], [D, 1]) for l in range(L)]
            sg1 = [load(g1[l], [D, 1]) for l in range(L)]
            sc1 = [load(c1[l], [D, 1]) for l in range(L)]
            sWf1 = [load(Wf1[l], [D, DF]) for l in range(L)]
            sbf1 = [load(bf1c[l], [D, 4]) for l in range(L)]
            sWf2 = [load(Wf2[l], [D, 4, D], "(c p) n -> p c n", p=D) for l in range(L)]
            sbf2 = [load(bf2[l], [D, 1]) for l in range(L)]
            sg2 = [load(g2[l], [D, 1]) for l in range(L)]
            sc2 = [load(c2[l], [D, 1]) for l in range(L)]
            snodeW = load(nodeW, [D, DN])
            snodeb = load(nodebB, [D, DN])
            sW1a = load(W1a, [D, D])
            sW1b = load(W1b, [D, D])
            sw1c = load(w1c, [1, D])
            sb1 = load(b1, [D, 1])
            sw2 = load(w2, [D, 1])
            steWa = load(teWa, [D, DE])
            steWb2 = load(teWb2, [D, 2 * DE])
            stewc = load(tewc, [1, DE])
            stewc2 = load(tewc2, [1, 2 * DE])
            steb2 = load(teb2, [2 * DE, 1])
            sb2c = load(b2col, [D, 1])
            sneg = load(negtvec, [1, S])
            sid = load(identD, [D, D])
            sidH = load(identH, [D, D])
            sones = load(ones_col, [D, 1])
            sonesr = load(ones_row, [1, D])
            sinvD = load(invD, [D, 1])
            sEselT = load(EselT, [D, 4, R], "(c p) j -> p c j", p=D)
            sEselTe = load(EselTe, [D, 4, R // 2], "(c p) j -> p c j", p=D)
            sEselTo = load(EselTo, [D, 4, R // 2], "(c p) j -> p c j", p=D)
            ssvec = load(svec, [1, R])
            ssvec_e = load(svec_e, [1, R // 2])
            ssvec_o = load(svec_o, [1, R // 2])
            su = load(u_lay, [D, NQ, RQ * 4], "q p j -> p q j")

            # ---------- gumbel noise precompute (only needs u) ----------
            gn_all = sg.tile([D, NQ, RQ * 4], f32, tag="gn_all")
            nc.vector.tensor_scalar(gn_all[:], su[:], 1e-8, 1.0 - 1e-8,
                                    OP.max, OP.min)
            nc.scalar.activation(gn_all[:], gn_all[:], AF.Ln, bias=zcol[:])
            nc.scalar.activation(gn_all[:], gn_all[:], AF.Ln, bias=zcol[:],
                                 scale=-1.0)
            nc.vector.tensor_scalar(gn_all[:], gn_all[:], -1.0, sb2c[:],
                                    OP.mult, OP.add)

            # ---------- backbone ----------
            for l in range(L):
                q_ps = pmm.tile([D, S], f32, tag="mm")
                nc.tensor.matmul(q_ps[:], r(sWq[l][:]), r(xT[:]), start=True, stop=True)
                qT = wk.tile([D, S], fr, tag="qT")
                nc.vector.tensor_scalar_add(qT[:], q_ps[:], sbq[l][:])
                k_ps = pmm.tile([D, S], f32, tag="mm")
                nc.tensor.matmul(k_ps[:], r(sWk[l][:]), r(xT[:]), start=True, stop=True)
                kT = wk.tile([D, S], fr, tag="kT")
                nc.vector.tensor_scalar_add(kT[:], k_ps[:], sbk[l][:])
                v_sb = wk.tile([D, 4, D], fr, tag="v")
                for j in range(4):
                    v_ps = psml.tile([D, D], f32, tag="sm")
                    nc.tensor.matmul(v_ps[:], xT[:, j * D:(j + 1) * D],
                                     sWv[l][:], start=True, stop=True)
                    nc.vector.tensor_copy(v_sb[:, j, :], v_ps[:])

                pj_ps = pacc.tile([D, S], f32, tag="acc")
                for h in range(H):
                    den_ps = psml.tile([1, S], f32, tag="sm")
                    oTh_ps = psml.tile([DH, S], f32, tag="sm")
                    for kc in range(4):
                        scT = pmm.tile([D, S], f32, tag="mm")
                        nc.tensor.matmul(
                            scT[:],
                            r(kT[32 * h:32 * h + 32, kc * D:(kc + 1) * D]),
                            r(qT[32 * h:32 * h + 32, :]),
                            start=True, stop=True, tile_position=(32 * h, 0))
                        P = hsp.tile([D, S], fr, tag="P")
                        nc.scalar.activation(P[:], scT[:], AF.Exp, bias=zcol[:], scale=SCALE)
                        nc.tensor.matmul(oTh_ps[:],
                                         r(v_sb[:, kc, 32 * h:32 * h + 32]),
                                         r(P[:]), start=(kc == 0), stop=(kc == 3))
                        nc.tensor.matmul(den_ps[:], r(sones[:]), r(P[:]),
                                         start=(kc == 0), stop=(kc == 3))
                    rd = wk.tile([1, S], fr, tag="rd")
                    nc.vector.reciprocal(rd[:], den_ps[:])
                    RDh_ps = psml.tile([DH, S], f32, tag="sm")
                    nc.tensor.matmul(RDh_ps[:], sonesr[0:1, 0:DH], rd[:],
                                     start=True, stop=True)
                    oTh_sb = wk.tile([DH, S], fr, tag="oTh_sb")
                    nc.vector.tensor_copy(oTh_sb[:], oTh_ps[:])
                    oTn = wk.tile([DH, S], fr, tag="oTn")
                    nc.vector.tensor_mul(oTn[:], oTh_sb[:], RDh_ps[:])
                    nc.tensor.matmul(pj_ps[:], sWo[l][:, h, :], oTn[:],
                                     start=(h == 0), stop=(h == 3))
                xa = wk.tile([D, S], fr, tag="xa")
                nc.vector.tensor_scalar_add(xa[:], pj_ps[:], sbo[l][:])
                nc.vector.tensor_add(xa[:], xa[:], xT[:])

                def layernorm(xin, gcol, ccol, outtag):
                    sq = wk.tile([D, S], fr, tag="sq")
                    nc.scalar.activation(sq[:], xin[:], AF.Square, bias=zcol[:])
                    mu_ps = psml.tile([1, S], f32, tag="sm")
                    nc.tensor.matmul(mu_ps[:], r(sinvD[:]), r(xin[:]), start=True, stop=True)
                    m2_ps = psml.tile([1, S], f32, tag="sm")
                    nc.tensor.matmul(m2_ps[:], r(sinvD[:]), r(sq[:]), start=True, stop=True)
                    mu = wk.tile([1, S], fr, tag="mu_sb")
                    nc.vector.tensor_copy(mu[:], mu_ps[:])
                    var = wk.tile([1, S], fr, tag="var")
                    nc.vector.tensor_mul(var[:], mu[:], mu[:])
                    nc.vector.tensor_sub(var[:], m2_ps[:], var[:])
                    sd = wk.tile([1, S], fr, tag="sd")
                    nc.scalar.activation(sd[:], var[:], AF.Sqrt, bias=ecol[:])
                    rstd = wk.tile([1, S], fr, tag="rstd")
                    nc.vector.reciprocal(rstd[:], sd[:])
                    MB = pmm.tile([D, S], f32, tag="mm")
                    nc.tensor.matmul(MB[:], r(sonesr[:]), r(mu[:]), start=True, stop=True)
                    RB = pmm.tile([D, S], f32, tag="mm")
                    nc.tensor.matmul(RB[:], r(sonesr[:]), r(rstd[:]), start=True, stop=True)
                    xout = wk.tile([D, S], fr, tag=outtag)
                    nc.vector.tensor_sub(xout[:], xin[:], MB[:])
                    nc.vector.tensor_mul(xout[:], xout[:], RB[:])
                    nc.vector.tensor_scalar(xout[:], xout[:], gcol[:], ccol[:],
                                            OP.mult, OP.add)
                    return xout

                xm = layernorm(xa, sg1[l], sc1[l], "xm")

                f2_ps = pacc.tile([D, S], f32, tag="acc")
                for j in range(4):
                    h_ps = pmm.tile([D, S], f32, tag="mm")
                    nc.tensor.matmul(h_ps[:], r(sWf1[l][:, j * D:(j + 1) * D]),
                                     r(xm[:]), start=True, stop=True)
                    hT = hsp.tile([D, S], fr, tag="hT")
                    nc.scalar.activation(hT[:], h_ps[:], GELU, bias=sbf1[l][:, j:j + 1])
                    nc.tensor.matmul(f2_ps[:], r(sWf2[l][:, j, :]), r(hT[:]),
                                     start=(j == 0), stop=(j == 3))
                xb = wk.tile([D, S], fr, tag="xb")
                nc.vector.tensor_scalar_add(xb[:], f2_ps[:], sbf2[l][:])
                nc.vector.tensor_add(xb[:], xb[:], xm[:])
                xT = layernorm(xb, sg2[l], sc2[l], "xT_out")

            # ---------- node head ----------
            for j in range(4):
                n_ps = psml.tile([D, DN], f32, tag="sm")
                nc.tensor.matmul(n_ps[:], xT[:, j * D:(j + 1) * D], snodeW[:],
                                 start=True, stop=True)
                n_sb = wk.tile([D, DN], f32, tag="n_sb")
                nc.vector.tensor_add(n_sb[:], n_ps[:], snodeb[:])
                nc.gpsimd.dma_start(node_out[j * D:(j + 1) * D, :], n_sb[:])

            # ---------- edge setup ----------
            C_ps = pmm.tile([D, S], f32, tag="mm")
            nc.tensor.matmul(C_ps[:], r(sW1b[:]), r(xT[:]), start=True, stop=False)
            nc.tensor.matmul(C_ps[:], r(sw1c[:]), r(sneg[:]), start=False, stop=True)
            C_sb = sg.tile([D, S], f32, tag="C_sb")
            nc.scalar.copy(C_sb[:], C_ps[:])

            CT2_ps = pmm.tile([D, S], f32, tag="mm")
            nc.tensor.matmul(CT2_ps[:], r(steWb2[:]), r(xT[:]), start=True, stop=False)
            nc.tensor.matmul(CT2_ps[:], r(stewc2[:]), r(sneg[:]), start=False, stop=True)
            CT2_sb = sg.tile([D, S], f32, tag="CT2_sb")
            nc.scalar.copy(CT2_sb[:], CT2_ps[:])

            AWsc = sg.tile([D, 4, D], fr, tag="AWsc")
            ATe = sg.tile([D, 4, DE], fr, tag="ATe")
            for tcn in range(4):
                aw_ps = psml.tile([D, D], f32, tag="sm")
                nc.tensor.matmul(aw_ps[:], xT[:, tcn * D:(tcn + 1) * D], sW1a[:],
                                 start=True, stop=True)
                nc.vector.tensor_copy(AWsc[:, tcn, :], aw_ps[:])
                at_ps = psml.tile([D, DE], f32, tag="sm")
                nc.tensor.matmul(at_ps[:], xT[:, tcn * D:(tcn + 1) * D], steWa[:],
                                 start=True, stop=True)
                nc.vector.tensor_copy(ATe[:, tcn, :], at_ps[:])

            rvT_ps = psml.tile([R, D], f32, tag="sm")
            for tcn in range(4):
                nc.tensor.matmul(rvT_ps[:], sEselT[:, tcn, :], AWsc[:, tcn, :],
                                 start=(tcn == 0), stop=False)
            nc.tensor.matmul(rvT_ps[:], ssvec[:], sw1c[:], start=False, stop=True)
            rvT_sb = wk.tile([R, D], fr, tag="rvT_sb")
            nc.vector.tensor_copy(rvT_sb[:], rvT_ps[:])
            rvsc_ps = psml.tile([D, R], fr, tag="sm")
            nc.tensor.matmul(rvsc_ps[:], rvT_sb[:], sid[0:R, 0:R],
                             is_transpose=True, start=True, stop=True)
            RVsc = sg.tile([D, R], f32, tag="RVsc")
            nc.vector.tensor_scalar_add(RVsc[:], rvsc_ps[:], sb1[:])

            rvte2_raw = sg.tile([2 * DE, R // 2], f32, tag="rvte2_raw")
            for half, (esel, sv) in enumerate(
                    [(sEselTe, ssvec_e), (sEselTo, ssvec_o)]):
                rvh_ps = psml.tile([R // 2, DE], f32, tag="sm")
                for tcn in range(4):
                    nc.tensor.matmul(rvh_ps[:], esel[:, tcn, :], ATe[:, tcn, :],
                                     start=(tcn == 0), stop=False)
                nc.tensor.matmul(rvh_ps[:], sv[:], stewc[:], start=False, stop=True)
                rvh_sb = wk.tile([R // 2, DE], fr, tag="rvh_sb")
                nc.vector.tensor_copy(rvh_sb[:], rvh_ps[:])
                rvt_ps = psml.tile([DE, R // 2], fr, tag="sm")
                nc.tensor.matmul(rvt_ps[:], rvh_sb[:], sid[0:R // 2, 0:R // 2],
                                 is_transpose=True, start=True, stop=True)
                rvt_sb = wk.tile([DE, R // 2], f32, tag="rvt_sb")
                nc.vector.tensor_copy(rvt_sb[:], rvt_ps[:])
                nc.sync.dma_start(rvte2_raw[half * DE:(half + 1) * DE, :], rvt_sb[:])
            RVte2 = sg.tile([2 * DE, R // 2], f32, tag="RVte2")
            nc.vector.tensor_scalar_add(RVte2[:], rvte2_raw[:], steb2[:])

            # ---------- per-quarter: logits -> gates -> gated features ----------
            import concourse.bass as bassmod
            for q in range(NQ):
                LGT_full = pacc.tile([D, S], f32, tag="acc", name="LGT")
                LGT = LGT_full[:, 0:RQ * 4]
                for rr in range(RQ):
                    sl = q * RQ + rr
                    hs = hsp.tile([D, S], f16, tag="hs")
                    nc.scalar.activation(hs[:], C_sb[:], GELU,
                                         bias=RVsc[:, sl:sl + 1])
                    for tcn in range(4):
                        nc.tensor.matmul(LGT[:, rr * 4 + tcn:rr * 4 + tcn + 1],
                                         hs[:, tcn * D:(tcn + 1) * D], sw2[:],
                                         start=True, stop=True)
                # gates for this batch, in [t-partition, (row,chunk)] layout
                gt = gtsp.tile([D, RQ * 4], f32, tag="gt_tmp")
                nc.vector.tensor_add(gt[:], gn_all[:, q, :], LGT[:])
                GTS = gtsp.tile([D, RQ * 4], f32, tag="GTS")
                nc.scalar.activation(GTS[:], gt[:], AF.Sigmoid, bias=zcol[:])
                nc.gpsimd.dma_start(gatesT_out[q, :, :], GTS[:])

                for pp in range(RQ // 2):
                    p_glob = q * (RQ // 2) + pp
                    re = 2 * pp          # even row index within quarter
                    t1 = fez.tile([2 * DE, S], f16, tag="t1")
                    nc.vector.tensor_scalar_add(t1[:], CT2_sb[:],
                                                RVte2[:, p_glob:p_glob + 1])
                    tr_ps = psml.tile([D, 4, D], f16, tag="sm")
                    for tcn in range(4):
                        nc.tensor.matmul(tr_ps[:, tcn, :],
                                         t1[:, tcn * D:(tcn + 1) * D], sidH[:],
                                         is_transpose=True, start=True, stop=True)
                    feats = fez.tile([D, 2, 4, DE], f32, tag="feats")
                    gts_ap = GTS[:]
                    bcast = bassmod.AP(
                        tensor=gts_ap.tensor,
                        offset=gts_ap.offset + re * 4,
                        ap=[list(gts_ap.ap[0]), [4, 2], [1, 4], [0, DE]],
                    )
                    nc.vector.tensor_tensor(
                        feats[:],
                        tr_ps[:].rearrange("p c (h f) -> p h c f", h=2),
                        bcast, OP.mult)
                    dst = feats_out[2 * p_glob:2 * p_glob + 2, :, :].rearrange(
                        "h (c p) f -> p h c f", p=D)
                    nc.gpsimd.dma_start(dst, feats[:])

    nc.compile()
    return nc


def _host_prep(token_ids, u, emb, Wqkv, bqkv, Wo, bo, ln1_g, ln1_b, Wff1, bff1,
               Wff2, bff2, ln2_g, ln2_b, node_W, node_b, sc_W1, sc_b1, sc_W2,
               sc_b2, te_W, te_b):
    f = np.float32
    a = lambda x: np.ascontiguousarray(x, dtype=f)
    x0 = emb[np.asarray(token_ids).astype(np.int64)].astype(f) + _pos_encoding()
    shared = {"x0T": a(x0.T)}
    for l in range(L):
        shared[f"Wq{l}"] = a(Wqkv[l][:, 0:D])
        shared[f"Wk{l}"] = a(Wqkv[l][:, D:2 * D])
        shared[f"Wv{l}"] = a(Wqkv[l][:, 2 * D:3 * D])
        shared[f"bq{l}"] = a(bqkv[l][0:D].reshape(D, 1))
        shared[f"bk{l}"] = a(bqkv[l][D:2 * D].reshape(D, 1))
        shared[f"Wo{l}"] = a(Wo[l].reshape(H, DH, D).transpose(1, 0, 2))
        bv = bqkv[l][2 * D:3 * D]
        shared[f"bo{l}"] = a((bo[l] + bv @ Wo[l]).reshape(D, 1))
        shared[f"g1{l}"] = a(ln1_g[l].reshape(D, 1))
        shared[f"c1{l}"] = a(ln1_b[l].reshape(D, 1))
        shared[f"Wf1{l}"] = a(Wff1[l])
        shared[f"bf1c{l}"] = a(bff1[l].reshape(4, D).T)
        shared[f"Wf2{l}"] = a(Wff2[l])
        shared[f"bf2{l}"] = a(bff2[l].reshape(D, 1))
        shared[f"g2{l}"] = a(ln2_g[l].reshape(D, 1))
        shared[f"c2{l}"] = a(ln2_b[l].reshape(D, 1))
    shared["nodeW"] = a(node_W)
    shared["nodebB"] = a(np.broadcast_to(node_b, (D, DN)))
    shared["W1a"] = a(sc_W1[0:D])
    shared["W1b"] = a(sc_W1[D:2 * D])
    shared["w1c"] = a(sc_W1[2 * D:2 * D + 1])
    shared["b1"] = a(sc_b1.reshape(D, 1))
    shared["w2"] = np.ascontiguousarray(sc_W2, dtype=np.float16)
    shared["identH"] = np.eye(D, dtype=np.float16)
    shared["teWa"] = a(te_W[0:D])
    teWb = te_W[D:2 * D]
    shared["teWb2"] = a(np.concatenate([teWb, teWb], axis=1))
    shared["tewc"] = a(te_W[2 * D:2 * D + 1])
    shared["tewc2"] = a(np.concatenate([te_W[2 * D:2 * D + 1]] * 2, axis=1))
    shared["teb2"] = a(np.concatenate([te_b, te_b]).reshape(2 * DE, 1))
    shared["b2col"] = a(np.full((D, 1), np.asarray(sc_b2).reshape(-1)[0]))
    shared["negtvec"] = a((-np.arange(S, dtype=f) / S).reshape(1, S))
    shared["identD"] = a(np.eye(D))
    shared["ones_col"] = a(np.ones((D, 1)))
    shared["ones_row"] = a(np.ones((1, D)))
    shared["invD"] = a(np.full((D, 1), 1.0 / D))

    u_grid = np.full((S, S), 0.5, dtype=f)
    u_grid[_SRC[_KEEP], _TGT[_KEEP]] = np.asarray(u, dtype=f)

    per_core = []
    for c in range(NC):
        s0 = c * R
        esel = np.zeros((S, R), dtype=f)
        esel[s0 + np.arange(R), np.arange(R)] = 1.0
        ug = u_grid[s0:s0 + R].reshape(NQ, RQ, 4, D)
        pc = {
            "EselT": esel,
            "EselTe": a(esel[:, 0::2]),
            "EselTo": a(esel[:, 1::2]),
            "svec": a(((s0 + np.arange(R)) / S).reshape(1, R)),
            "svec_e": a(((s0 + np.arange(0, R, 2)) / S).reshape(1, R // 2)),
            "svec_o": a(((s0 + np.arange(1, R, 2)) / S).reshape(1, R // 2)),
            "u_lay": a(ug.transpose(0, 3, 1, 2).reshape(NQ, D, RQ * 4)),
        }
        per_core.append(pc)
    return shared, per_core


def _assemble(results):
    node_features = np.asarray(results[0]["node_out"], dtype=np.float32)
    feats_grid = np.concatenate(
        [np.asarray(r["feats_out"]) for r in results], axis=0)
    edge_feats = np.ascontiguousarray(
        feats_grid.reshape(S * S, DE)[_KEEP]).astype(np.float32)
    gates_rows = []
    for rres in results:
        gt = np.asarray(rres["gatesT_out"])  # [NQ, D, R]
        gates_rows.append(
            gt.reshape(NQ, D, RQ, 4).transpose(0, 2, 3, 1).reshape(R, S))
    gates_grid = np.concatenate(gates_rows, axis=0)
    edge_gates = np.ascontiguousarray(
        gates_grid.reshape(S * S)[_KEEP]).astype(np.float32)
    return node_features, edge_feats, _EDGE_INDEX.copy(), edge_gates


def kernel(**inputs):
    from concourse.bass_utils import run_bass_kernel_spmd

    if "hw" not in _BUILT:
        _BUILT["hw"] = build_module("hw")
    nc = _BUILT["hw"]
    shared, per_core = _host_prep(**inputs)
    in_maps = [{**shared, **pc} for pc in per_core]
    res = run_bass_kernel_spmd(nc, in_maps, core_ids=list(range(NC)))
    return _assemble(res.results)


# revision 19
# speedup vs baseline: 1.3502x; 1.1469x over previous
"""Trainium2 Bass kernel for nn_DifferentiableConstructor (gnn_message_passing).

Strategy (8 cores, SPMD — identical NEFF, per-core input data):
  * Host: embedding gather + positional encoding -> x0T [D,S] (replicated).
  * Device (every core): 2-layer transformer backbone in transposed layout
    [D partitions, S free]; node head; then its 64 source-row slice of the
    (full SxS grid) edge computation:
      - edge scorer:  hs = gelu(C + rowvec_s)  with C = W1b^T xT + w1c x (-t/S)
        shared across rows, rowvec per-partition bias; logits via PE with hs
        as the stationary operand -> t-partitioned logit columns.
      - gumbel-sigmoid gates computed in the same t-partitioned layout
        (u pre-laid-out by host).
      - gated edge features: te-branch in doubled [2x64f, S] layout, bias add,
        PE transpose, one broadcast multiply by the gate columns, DMA out.
  * Host: concatenate 8 slices, drop the diagonal, emit static edge_index.
"""
import numpy as np

S = 512
V = 32000
D = 128
H = 4
DH = 32
L = 2
DN = 64
DE = 64
DF = 4 * D
E = S * S - S
NC = 8
R = S // NC          # 64 source rows per core
NQ = 2               # row-batches per core
RQ = R // NQ         # 32 rows per batch

_SRC = np.repeat(np.arange(S), S)
_TGT = np.tile(np.arange(S), S)
_KEEP = _SRC != _TGT
_EDGE_INDEX = np.stack([_SRC[_KEEP], _TGT[_KEEP]]).astype(np.int32)

_BUILT = {}


def _pos_encoding():
    pos = np.arange(S, dtype=np.float32)[:, None]
    div = np.exp(np.arange(0, D, 2, dtype=np.float32) * (-np.log(10000.0) / D))
    pe = np.zeros((S, D), dtype=np.float32)
    pe[:, 0::2] = np.sin(pos * div)
    pe[:, 1::2] = np.cos(pos * div)
    return pe


def build_module(gelu_mode="hw"):
    """Build the Bass module (same program for all 8 cores).

    gelu_mode="sim" replaces Gelu with Identity (CoreSim does not implement
    Gelu); the numpy model in sim tests does the same.
    """
    import concourse.bass as bass
    import concourse.tile as tile
    from concourse import bacc, mybir

    f32 = mybir.dt.float32
    fr = mybir.dt.float32r
    f16 = mybir.dt.float16
    AF = mybir.ActivationFunctionType
    OP = mybir.AluOpType
    GELU = AF.Identity if gelu_mode == "sim" else AF.Gelu

    nc = bacc.Bacc("TRN2", target_bir_lowering=False, debug=False)

    def din(name, shape, dt=f32):
        return nc.dram_tensor(name, shape, dt, kind="ExternalInput")

    x0T = din("x0T", [D, S], f16)
    Wq, Wk, Wv, bq, bk, Wo, bo = [], [], [], [], [], [], []
    g1, c1, Wf1, bf1c, Wf2, bf2, g2, c2 = [], [], [], [], [], [], [], []
    for l in range(L):
        Wq.append(din(f"Wq{l}", [D, D], f16))
        Wk.append(din(f"Wk{l}", [D, D], f16))
        Wv.append(din(f"Wv{l}", [D, D], f16))
        bq.append(din(f"bq{l}", [D, 1]))
        bk.append(din(f"bk{l}", [D, 1]))
        Wo.append(din(f"Wo{l}", [DH, H, D], f16))
        bo.append(din(f"bo{l}", [D, 1]))
        g1.append(din(f"g1{l}", [D, 1]))
        c1.append(din(f"c1{l}", [D, 1]))
        Wf1.append(din(f"Wf1{l}", [D, DF], f16))
        bf1c.append(din(f"bf1c{l}", [D, 4]))
        Wf2.append(din(f"Wf2{l}", [DF, D], f16))
        bf2.append(din(f"bf2{l}", [D, 1]))
        g2.append(din(f"g2{l}", [D, 1]))
        c2.append(din(f"c2{l}", [D, 1]))
    nodeW = din("nodeW", [D, DN], f16)
    nodebB = din("nodebB", [D, DN])
    W1a = din("W1a", [D, D], f16)
    W1b = din("W1b", [D, D], f16)
    w1c = din("w1c", [1, D], f16)
    b1 = din("b1", [D, 1])
    w2 = din("w2", [D, 1], f16)
    identH = din("identH", [D, D], f16)
    teWa = din("teWa", [D, DE], f16)
    teWb2 = din("teWb2", [D, 2 * DE], f16)
    tewc = din("tewc", [1, DE], f16)
    tewc2 = din("tewc2", [1, 2 * DE], f16)
    teb2 = din("teb2", [2 * DE, 1])
    b2col = din("b2col", [D, 1])
    negtvec = din("negtvec", [1, S], f16)
    identD = din("identD", [D, D], fr)
    ones_col = din("ones_col", [D, 1], f16)
    ones_row = din("ones_row", [1, D], f16)
    invD = din("invD", [D, 1], f16)
    # per-core data
    EselT = din("EselT", [S, R], f16)
    EselTe = din("EselTe", [S, R // 2], f16)
    EselTo = din("EselTo", [S, R // 2], f16)
    svec = din("svec", [1, R], f16)
    svec_e = din("svec_e", [1, R // 2], f16)
    svec_o = din("svec_o", [1, R // 2], f16)
    u_lay = din("u_lay", [NQ, D, RQ * 4])

    node_out = nc.dram_tensor("node_out", [S, DN], f32, kind="ExternalOutput")
    feats_out = nc.dram_tensor("feats_out", [R, S, DE], f32, kind="ExternalOutput")
    gatesT_out = nc.dram_tensor("gatesT_out", [NQ, D, RQ * 4], f32, kind="ExternalOutput")

    SCALE = float(1.0 / np.sqrt(DH))

    def r(ap):
        return ap

    with tile.TileContext(nc) as tc:
        with (
            nc.allow_low_precision(reason="fp32r operand chain for PE fast fp32"),
            tc.tile_pool(name="singles", bufs=1) as sg,
            tc.tile_pool(name="work", bufs=3) as wk,
            tc.tile_pool(name="hsp", bufs=3) as hsp,
            tc.tile_pool(name="gts", bufs=2) as gtsp,
            tc.tile_pool(name="fez", bufs=3) as fez,
            tc.tile_pool(name="pmm", bufs=2, space="PSUM") as pmm,
            tc.tile_pool(name="pacc", bufs=2, space="PSUM") as pacc,
            tc.tile_pool(name="psml", bufs=3, space="PSUM") as psml,
        ):
            # ---------- load constants/weights ----------
            _load_engines = [nc.sync, nc.scalar, nc.gpsimd]
            _load_ctr = [0]

            def load(dram, shape, rearr=None, **axes):
                t = sg.tile(shape, dram.dtype, tag=dram.name)
                src = dram[:]
                if rearr is not None:
                    src = src.rearrange(rearr, **axes)
                eng = _load_engines[_load_ctr[0] % len(_load_engines)]
                _load_ctr[0] += 1
                eng.dma_start(t[:], src)
                return t

            zcol = sg.tile([D, 1], f32, tag="zcol")
            nc.vector.memset(zcol[:], 0.0)
            ecol = sg.tile([1, 1], f32, tag="ecol")
            nc.vector.memset(ecol[:], 1e-5)
            xT = sg.tile([D, S], f16, tag="xT0")
            nc.sync.dma_start(xT[:], x0T[:])
            sWq = [load(Wq[l], [D, D]) for l in range(L)]
            sWk = [load(Wk[l], [D, D]) for l in range(L)]
            sWv = [load(Wv[l], [D, D]) for l in range(L)]
            sbq = [load(bq[l], [D, 1]) for l in range(L)]
            sbk = [load(bk[l], [D, 1]) for l in range(L)]
            sWo = [load(Wo[l], [DH, H, D]) for l in range(L)]
            sbo = [load(bo[l], [D, 1]) for l in range(L)]
            sg1 = [load(g1[l], [D, 1]) for l in range(L)]
            sc1 = [load(c1[l], [D, 1]) for l in range(L)]
            sWf1 = [load(Wf1[l], [D, DF]) for l in range(L)]
            sbf1 = [load(bf1c[l], [D, 4]) for l in range(L)]
            sWf2 = [load(Wf2[l], [D, 4, D], "(c p) n -> p c n", p=D) for l in range(L)]
            sbf2 = [load(bf2[l], [D, 1]) for l in range(L)]
            sg2 = [load(g2[l], [D, 1]) for l in range(L)]
            sc2 = [load(c2[l], [D, 1]) for l in range(L)]
            snodeW = load(nodeW, [D, DN])
            snodeb = load(nodebB, [D, DN])
            sW1a = load(W1a, [D, D])
            sW1b = load(W1b, [D, D])
            sw1c = load(w1c, [1, D])
            sb1 = load(b1, [D, 1])
            sw2 = load(w2, [D, 1])
            steWa = load(teWa, [D, DE])
            steWb2 = load(teWb2, [D, 2 * DE])
            stewc = load(tewc, [1, DE])
            stewc2 = load(tewc2, [1, 2 * DE])
            steb2 = load(teb2, [2 * DE, 1])
            sb2c = load(b2col, [D, 1])
            sneg = load(negtvec, [1, S])
            sid = load(identD, [D, D])
            sidH = load(identH, [D, D])
            sones = load(ones_col, [D, 1])
            sonesr = load(ones_row, [1, D])
            sinvD = load(invD, [D, 1])
            sEselT = load(EselT, [D, 4, R], "(c p) j -> p c j", p=D)
            sEselTe = load(EselTe, [D, 4, R // 2], "(c p) j -> p c j", p=D)
            sEselTo = load(EselTo, [D, 4, R // 2], "(c p) j -> p c j", p=D)
            ssvec = load(svec, [1, R])
            ssvec_e = load(svec_e, [1, R // 2])
            ssvec_o = load(svec_o, [1, R // 2])
            su = load(u_lay, [D, NQ, RQ * 4], "q p j -> p q j")

            # ---------- gumbel noise precompute (only needs u) ----------
            gn_all = sg.tile([D, NQ, RQ * 4], f32, tag="gn_all")
            nc.vector.tensor_scalar(gn_all[:], su[:], 1e-8, 1.0 - 1e-8,
                                    OP.max, OP.min)
            nc.scalar.activation(gn_all[:], gn_all[:], AF.Ln, bias=zcol[:])
            nc.scalar.activation(gn_all[:], gn_all[:], AF.Ln, bias=zcol[:],
                                 scale=-1.0)
            nc.vector.tensor_scalar(gn_all[:], gn_all[:], -1.0, sb2c[:],
                                    OP.mult, OP.add)

            # ---------- backbone ----------
            for l in range(L):
                q_ps = pmm.tile([D, S], f32, tag="mm")
                nc.tensor.matmul(q_ps[:], r(sWq[l][:]), r(xT[:]), start=True, stop=True)
                qT = wk.tile([D, S], f16, tag="qT")
                nc.vector.tensor_scalar_add(qT[:], q_ps[:], sbq[l][:])
                k_ps = pmm.tile([D, S], f32, tag="mm")
                nc.tensor.matmul(k_ps[:], r(sWk[l][:]), r(xT[:]), start=True, stop=True)
                kT = wk.tile([D, S], f16, tag="kT")
                nc.vector.tensor_scalar_add(kT[:], k_ps[:], sbk[l][:])
                v_sb = wk.tile([D, 4, D], f16, tag="v")
                for j in range(4):
                    v_ps = psml.tile([D, D], f32, tag="sm")
                    nc.tensor.matmul(v_ps[:], xT[:, j * D:(j + 1) * D],
                                     sWv[l][:], start=True, stop=True)
                    nc.vector.tensor_copy(v_sb[:, j, :], v_ps[:])

                pj_ps = pacc.tile([D, S], f32, tag="acc")
                for h in range(H):
                    den_ps = psml.tile([1, S], f32, tag="sm")
                    oTh_ps = psml.tile([DH, S], f32, tag="sm")
                    for kc in range(4):
                        scT = pmm.tile([D, S], f32, tag="mm")
                        nc.tensor.matmul(
                            scT[:],
                            r(kT[32 * h:32 * h + 32, kc * D:(kc + 1) * D]),
                            r(qT[32 * h:32 * h + 32, :]),
                            start=True, stop=True, tile_position=(32 * h, 0))
                        P = hsp.tile([D, S], f16, tag="P")
                        nc.scalar.activation(P[:], scT[:], AF.Exp, bias=zcol[:], scale=SCALE)
                        nc.tensor.matmul(oTh_ps[:],
                                         r(v_sb[:, kc, 32 * h:32 * h + 32]),
                                         r(P[:]), start=(kc == 0), stop=(kc == 3))
                        nc.tensor.matmul(den_ps[:], r(sones[:]), r(P[:]),
                                         start=(kc == 0), stop=(kc == 3))
                    rd = wk.tile([1, S], f16, tag="rd")
                    nc.vector.reciprocal(rd[:], den_ps[:])
                    RDh_ps = psml.tile([DH, S], f32, tag="sm")
                    nc.tensor.matmul(RDh_ps[:], sonesr[0:1, 0:DH], rd[:],
                                     start=True, stop=True)
                    oTh_sb = wk.tile([DH, S], f16, tag="oTh_sb")
                    nc.vector.tensor_copy(oTh_sb[:], oTh_ps[:])
                    oTn = wk.tile([DH, S], f16, tag="oTn")
                    nc.vector.tensor_mul(oTn[:], oTh_sb[:], RDh_ps[:])
                    nc.tensor.matmul(pj_ps[:], sWo[l][:, h, :], oTn[:],
                                     start=(h == 0), stop=(h == 3))
                xa = wk.tile([D, S], f16, tag="xa")
                nc.vector.tensor_scalar_add(xa[:], pj_ps[:], sbo[l][:])
                nc.vector.tensor_add(xa[:], xa[:], xT[:])

                def layernorm(xin, gcol, ccol, outtag):
                    sq = wk.tile([D, S], f16, tag="sq")
                    nc.scalar.activation(sq[:], xin[:], AF.Square, bias=zcol[:])
                    mu_ps = psml.tile([1, S], f32, tag="sm")
                    nc.tensor.matmul(mu_ps[:], r(sinvD[:]), r(xin[:]), start=True, stop=True)
                    m2_ps = psml.tile([1, S], f32, tag="sm")
                    nc.tensor.matmul(m2_ps[:], r(sinvD[:]), r(sq[:]), start=True, stop=True)
                    mu = wk.tile([1, S], f16, tag="mu_sb")
                    nc.vector.tensor_copy(mu[:], mu_ps[:])
                    var = wk.tile([1, S], f32, tag="var")
                    nc.vector.tensor_mul(var[:], mu[:], mu[:])
                    nc.vector.tensor_sub(var[:], m2_ps[:], var[:])
                    sd = wk.tile([1, S], f32, tag="sd")
                    nc.scalar.activation(sd[:], var[:], AF.Sqrt, bias=ecol[:])
                    rstd = wk.tile([1, S], f16, tag="rstd")
                    nc.vector.reciprocal(rstd[:], sd[:])
                    MB = pmm.tile([D, S], f32, tag="mm")
                    nc.tensor.matmul(MB[:], r(sonesr[:]), r(mu[:]), start=True, stop=True)
                    RB = pmm.tile([D, S], f32, tag="mm")
                    nc.tensor.matmul(RB[:], r(sonesr[:]), r(rstd[:]), start=True, stop=True)
                    xout = wk.tile([D, S], f16, tag=outtag)
                    nc.vector.tensor_sub(xout[:], xin[:], MB[:])
                    nc.vector.tensor_mul(xout[:], xout[:], RB[:])
                    nc.vector.tensor_scalar(xout[:], xout[:], gcol[:], ccol[:],
                                            OP.mult, OP.add)
                    return xout

                xm = layernorm(xa, sg1[l], sc1[l], "xm")

                f2_ps = pacc.tile([D, S], f32, tag="acc")
                for j in range(4):
                    h_ps = pmm.tile([D, S], f32, tag="mm")
                    nc.tensor.matmul(h_ps[:], r(sWf1[l][:, j * D:(j + 1) * D]),
                                     r(xm[:]), start=True, stop=True)
                    hT = hsp.tile([D, S], f16, tag="hT")
                    nc.scalar.activation(hT[:], h_ps[:], GELU, bias=sbf1[l][:, j:j + 1])
                    nc.tensor.matmul(f2_ps[:], r(sWf2[l][:, j, :]), r(hT[:]),
                                     start=(j == 0), stop=(j == 3))
                xb = wk.tile([D, S], f16, tag="xb")
                nc.vector.tensor_scalar_add(xb[:], f2_ps[:], sbf2[l][:])
                nc.vector.tensor_add(xb[:], xb[:], xm[:])
                xT = layernorm(xb, sg2[l], sc2[l], "xT_out")

            # ---------- node head ----------
            for j in range(4):
                n_ps = psml.tile([D, DN], f32, tag="sm")
                nc.tensor.matmul(n_ps[:], xT[:, j * D:(j + 1) * D], snodeW[:],
                                 start=True, stop=True)
                n_sb = wk.tile([D, DN], f32, tag="n_sb")
                nc.vector.tensor_add(n_sb[:], n_ps[:], snodeb[:])
                nc.gpsimd.dma_start(node_out[j * D:(j + 1) * D, :], n_sb[:])

            # ---------- edge setup ----------
            C_ps = pmm.tile([D, S], f32, tag="mm")
            nc.tensor.matmul(C_ps[:], r(sW1b[:]), r(xT[:]), start=True, stop=False)
            nc.tensor.matmul(C_ps[:], r(sw1c[:]), r(sneg[:]), start=False, stop=True)
            C_sb = sg.tile([D, S], f32, tag="C_sb")
            nc.scalar.copy(C_sb[:], C_ps[:])

            CT2_ps = pmm.tile([D, S], f32, tag="mm")
            nc.tensor.matmul(CT2_ps[:], r(steWb2[:]), r(xT[:]), start=True, stop=False)
            nc.tensor.matmul(CT2_ps[:], r(stewc2[:]), r(sneg[:]), start=False, stop=True)
            CT2_sb = sg.tile([D, S], f32, tag="CT2_sb")
            nc.scalar.copy(CT2_sb[:], CT2_ps[:])

            AWsc = sg.tile([D, 4, D], f16, tag="AWsc")
            ATe = sg.tile([D, 4, DE], f16, tag="ATe")
            for tcn in range(4):
                aw_ps = psml.tile([D, D], f32, tag="sm")
                nc.tensor.matmul(aw_ps[:], xT[:, tcn * D:(tcn + 1) * D], sW1a[:],
                                 start=True, stop=True)
                nc.vector.tensor_copy(AWsc[:, tcn, :], aw_ps[:])
                at_ps = psml.tile([D, DE], f32, tag="sm")
                nc.tensor.matmul(at_ps[:], xT[:, tcn * D:(tcn + 1) * D], steWa[:],
                                 start=True, stop=True)
                nc.vector.tensor_copy(ATe[:, tcn, :], at_ps[:])

            rvT_ps = psml.tile([R, D], f32, tag="sm")
            for tcn in range(4):
                nc.tensor.matmul(rvT_ps[:], sEselT[:, tcn, :], AWsc[:, tcn, :],
                                 start=(tcn == 0), stop=False)
            nc.tensor.matmul(rvT_ps[:], ssvec[:], sw1c[:], start=False, stop=True)
            rvT_sb = wk.tile([R, D], f16, tag="rvT_sb")
            nc.vector.tensor_copy(rvT_sb[:], rvT_ps[:])
            rvsc_ps = psml.tile([D, R], f16, tag="sm")
            nc.tensor.matmul(rvsc_ps[:], rvT_sb[:], sidH[0:R, 0:R],
                             is_transpose=True, start=True, stop=True)
            RVsc = sg.tile([D, R], f32, tag="RVsc")
            nc.vector.tensor_scalar_add(RVsc[:], rvsc_ps[:], sb1[:])

            rvte2_raw = sg.tile([2 * DE, R // 2], f32, tag="rvte2_raw")
            for half, (esel, sv) in enumerate(
                    [(sEselTe, ssvec_e), (sEselTo, ssvec_o)]):
                rvh_ps = psml.tile([R // 2, DE], f32, tag="sm")
                for tcn in range(4):
                    nc.tensor.matmul(rvh_ps[:], esel[:, tcn, :], ATe[:, tcn, :],
                                     start=(tcn == 0), stop=False)
                nc.tensor.matmul(rvh_ps[:], sv[:], stewc[:], start=False, stop=True)
                rvh_sb = wk.tile([R // 2, DE], f16, tag="rvh_sb")
                nc.vector.tensor_copy(rvh_sb[:], rvh_ps[:])
                rvt_ps = psml.tile([DE, R // 2], f16, tag="sm")
                nc.tensor.matmul(rvt_ps[:], rvh_sb[:], sidH[0:R // 2, 0:R // 2],
                                 is_transpose=True, start=True, stop=True)
                rvt_sb = wk.tile([DE, R // 2], f32, tag="rvt_sb")
                nc.vector.tensor_copy(rvt_sb[:], rvt_ps[:])
                nc.sync.dma_start(rvte2_raw[half * DE:(half + 1) * DE, :], rvt_sb[:])
            RVte2 = sg.tile([2 * DE, R // 2], f32, tag="RVte2")
            nc.vector.tensor_scalar_add(RVte2[:], rvte2_raw[:], steb2[:])

            # ---------- per-quarter: logits -> gates -> gated features ----------
            import concourse.bass as bassmod
            for q in range(NQ):
                LGT_full = pacc.tile([D, S], f32, tag="acc", name="LGT")
                LGT = LGT_full[:, 0:RQ * 4]
                for rr in range(RQ):
                    sl = q * RQ + rr
                    hs = hsp.tile([D, S], f16, tag="hs")
                    nc.scalar.activation(hs[:], C_sb[:], GELU,
                                         bias=RVsc[:, sl:sl + 1])
                    for tcn in range(4):
                        nc.tensor.matmul(LGT[:, rr * 4 + tcn:rr * 4 + tcn + 1],
                                         hs[:, tcn * D:(tcn + 1) * D], sw2[:],
                                         start=True, stop=True)
                # gates for this batch, in [t-partition, (row,chunk)] layout
                gt = gtsp.tile([D, RQ * 4], f32, tag="gt_tmp")
                nc.vector.tensor_add(gt[:], gn_all[:, q, :], LGT[:])
                GTS = gtsp.tile([D, RQ * 4], f32, tag="GTS")
                nc.scalar.activation(GTS[:], gt[:], AF.Sigmoid, bias=zcol[:])
                nc.gpsimd.dma_start(gatesT_out[q, :, :], GTS[:])

                for pp in range(RQ // 2):
                    p_glob = q * (RQ // 2) + pp
                    re = 2 * pp          # even row index within quarter
                    t1 = fez.tile([2 * DE, S], f16, tag="t1")
                    nc.vector.tensor_scalar_add(t1[:], CT2_sb[:],
                                                RVte2[:, p_glob:p_glob + 1])
                    tr_ps = psml.tile([D, 4, D], f16, tag="sm")
                    for tcn in range(4):
                        nc.tensor.matmul(tr_ps[:, tcn, :],
                                         t1[:, tcn * D:(tcn + 1) * D], sidH[:],
                                         is_transpose=True, start=True, stop=True)
                    feats = fez.tile([D, 2, 4, DE], f32, tag="feats")
                    gts_ap = GTS[:]
                    bcast = bassmod.AP(
                        tensor=gts_ap.tensor,
                        offset=gts_ap.offset + re * 4,
                        ap=[list(gts_ap.ap[0]), [4, 2], [1, 4], [0, DE]],
                    )
                    nc.vector.tensor_tensor(
                        feats[:],
                        tr_ps[:].rearrange("p c (h f) -> p h c f", h=2),
                        bcast, OP.mult)
                    dst = feats_out[2 * p_glob:2 * p_glob + 2, :, :].rearrange(
                        "h (c p) f -> p h c f", p=D)
                    (nc.sync if pp % 2 == 0 else nc.gpsimd).dma_start(dst, feats[:])

    nc.compile()
    return nc


def _host_prep(token_ids, u, emb, Wqkv, bqkv, Wo, bo, ln1_g, ln1_b, Wff1, bff1,
               Wff2, bff2, ln2_g, ln2_b, node_W, node_b, sc_W1, sc_b1, sc_W2,
               sc_b2, te_W, te_b):
    f = np.float32
    a = lambda x: np.ascontiguousarray(x, dtype=f)
    h = lambda x: np.ascontiguousarray(x, dtype=np.float16)
    x0 = emb[np.asarray(token_ids).astype(np.int64)].astype(f) + _pos_encoding()
    shared = {"x0T": h(x0.T)}
    for l in range(L):
        shared[f"Wq{l}"] = h(Wqkv[l][:, 0:D])
        shared[f"Wk{l}"] = h(Wqkv[l][:, D:2 * D])
        shared[f"Wv{l}"] = h(Wqkv[l][:, 2 * D:3 * D])
        shared[f"bq{l}"] = a(bqkv[l][0:D].reshape(D, 1))
        shared[f"bk{l}"] = a(bqkv[l][D:2 * D].reshape(D, 1))
        shared[f"Wo{l}"] = h(Wo[l].reshape(H, DH, D).transpose(1, 0, 2))
        bv = bqkv[l][2 * D:3 * D]
        shared[f"bo{l}"] = a((bo[l] + bv @ Wo[l]).reshape(D, 1))
        shared[f"g1{l}"] = a(ln1_g[l].reshape(D, 1))
        shared[f"c1{l}"] = a(ln1_b[l].reshape(D, 1))
        shared[f"Wf1{l}"] = h(Wff1[l])
        shared[f"bf1c{l}"] = a(bff1[l].reshape(4, D).T)
        shared[f"Wf2{l}"] = h(Wff2[l])
        shared[f"bf2{l}"] = a(bff2[l].reshape(D, 1))
        shared[f"g2{l}"] = a(ln2_g[l].reshape(D, 1))
        shared[f"c2{l}"] = a(ln2_b[l].reshape(D, 1))
    shared["nodeW"] = h(node_W)
    shared["nodebB"] = a(np.broadcast_to(node_b, (D, DN)))
    shared["W1a"] = h(sc_W1[0:D])
    shared["W1b"] = h(sc_W1[D:2 * D])
    shared["w1c"] = h(sc_W1[2 * D:2 * D + 1])
    shared["b1"] = a(sc_b1.reshape(D, 1))
    shared["w2"] = np.ascontiguousarray(sc_W2, dtype=np.float16)
    shared["identH"] = np.eye(D, dtype=np.float16)
    shared["teWa"] = h(te_W[0:D])
    teWb = te_W[D:2 * D]
    shared["teWb2"] = h(np.concatenate([teWb, teWb], axis=1))
    shared["tewc"] = h(te_W[2 * D:2 * D + 1])
    shared["tewc2"] = h(np.concatenate([te_W[2 * D:2 * D + 1]] * 2, axis=1))
    shared["teb2"] = a(np.concatenate([te_b, te_b]).reshape(2 * DE, 1))
    shared["b2col"] = a(np.full((D, 1), np.asarray(sc_b2).reshape(-1)[0]))
    shared["negtvec"] = h((-np.arange(S, dtype=f) / S).reshape(1, S))
    shared["identD"] = a(np.eye(D))
    shared["ones_col"] = h(np.ones((D, 1)))
    shared["ones_row"] = h(np.ones((1, D)))
    shared["invD"] = h(np.full((D, 1), 1.0 / D))

    u_grid = np.full((S, S), 0.5, dtype=f)
    u_grid[_SRC[_KEEP], _TGT[_KEEP]] = np.asarray(u, dtype=f)

    per_core = []
    for c in range(NC):
        s0 = c * R
        esel = np.zeros((S, R), dtype=f)
        esel[s0 + np.arange(R), np.arange(R)] = 1.0
        ug = u_grid[s0:s0 + R].reshape(NQ, RQ, 4, D)
        pc = {
            "EselT": h(esel),
            "EselTe": h(esel[:, 0::2]),
            "EselTo": h(esel[:, 1::2]),
            "svec": h(((s0 + np.arange(R)) / S).reshape(1, R)),
            "svec_e": h(((s0 + np.arange(0, R, 2)) / S).reshape(1, R // 2)),
            "svec_o": h(((s0 + np.arange(1, R, 2)) / S).reshape(1, R // 2)),
            "u_lay": a(ug.transpose(0, 3, 1, 2).reshape(NQ, D, RQ * 4)),
        }
        per_core.append(pc)
    return shared, per_core


def _assemble(results):
    node_features = np.asarray(results[0]["node_out"], dtype=np.float32)
    feats_grid = np.concatenate(
        [np.asarray(r["feats_out"]) for r in results], axis=0)
    edge_feats = np.ascontiguousarray(
        feats_grid.reshape(S * S, DE)[_KEEP]).astype(np.float32)
    gates_rows = []
    for rres in results:
        gt = np.asarray(rres["gatesT_out"])  # [NQ, D, R]
        gates_rows.append(
            gt.reshape(NQ, D, RQ, 4).transpose(0, 2, 3, 1).reshape(R, S))
    gates_grid = np.concatenate(gates_rows, axis=0)
    edge_gates = np.ascontiguousarray(
        gates_grid.reshape(S * S)[_KEEP]).astype(np.float32)
    return node_features, edge_feats, _EDGE_INDEX.copy(), edge_gates


def kernel(**inputs):
    from concourse.bass_utils import run_bass_kernel_spmd

    if "hw" not in _BUILT:
        _BUILT["hw"] = build_module("hw")
    nc = _BUILT["hw"]
    shared, per_core = _host_prep(**inputs)
    in_maps = [{**shared, **pc} for pc in per_core]
    res = run_bass_kernel_spmd(nc, in_maps, core_ids=list(range(NC)))
    return _assemble(res.results)
